# revision 17
# baseline (speedup 1.0000x reference)
"""BasicTransformer Trainium2 kernel (Bass/Tile), data-parallel over batch on 8 cores.

Per batch b (one NeuronCore each):
    e   = embed[x[b]]                    (T, D)   indirect-DMA gather
    e^T, W^T via PE transposes (f32r) -> bf16 SBUF tiles
    q/k = W^T-stationary matmuls         PE bf16 -> fp8e4 (scores operands)
    v   = E^T-stationary matmuls         PE bf16, [t-part, d] layout
    s   = (q^T k)                        PE fp8 DoubleRow (2 k-subtiles/pass)
    p   = exp(s*SCALE - max*SCALE)       DVE rowmax + ACT exp (accum -> l)
    p^T via DMA-XBAR transpose           -> PT tiles [j-part, jc, i] bf16
    y   = (p @ v) * (1/l)                PE (PT stationary, V moving) +
                                         per-partition tensor_scalar (Pool)
    y^T via DMA-XBAR                     -> linear in [o-part, t] layout
    z   = relu(lw y + b), accum over t   PE + ACT
    out = sigmoid(clf . mean + clf_b)    PE + ACT

The attention inner loop is software-pipelined per 128-query chunk with
pv lagging scores by 2 and the linear by 3 chunks, keeping the PE dense
while the DVE/ACT softmax chain and the XBAR transposes run in the
shadow.  t-order inside the kernel is a fixed permutation of the true
t-order; the computation is permutation-invariant over t, so the final
(1,) output is unaffected.
"""

import math
import os

import numpy as np

B, T, D, VOCAB = 8, 2048, 512, 32000
P = 128
TC = T // P          # 16 t-chunks
DC = D // P          # 4 d-chunks
NB = T // 512        # 4 key blocks
SCALE = 1.0 / math.sqrt(D)
N_CORES = 8

_COMPILED = {}


def _build(iters=1, sdt_name=None):
    import concourse.bacc as bacc
    import concourse.mybir as mybir
    import concourse.tile as tile
    from concourse.masks import make_identity

    dt = mybir.dt
    if sdt_name is None:
        sdt_name = os.environ.get("KERNEL_SCORES_DTYPE", "bf16")
    sdt = {"f8": dt.float8e4, "bf16": dt.bfloat16, "mix": "mix"}[sdt_name]

    nc = bacc.Bacc("TRN2", target_bir_lowering=False, debug=False)

    x_d = nc.declare_dram_parameter("x", [T], dt.int32, isOutput=False)
    emb_d = nc.declare_dram_parameter("embed", [VOCAB + 1, D], dt.float32, isOutput=False)
    wq_d = nc.declare_dram_parameter("W_q", [D, D], dt.float32, isOutput=False)
    wk_d = nc.declare_dram_parameter("W_k", [D, D], dt.float32, isOutput=False)
    wv_d = nc.declare_dram_parameter("W_v", [D, D], dt.float32, isOutput=False)
    lw_d = nc.declare_dram_parameter("lin_w", [D, D], dt.float32, isOutput=False)
    lb_d = nc.declare_dram_parameter("lin_b", [D], dt.float32, isOutput=False)
    cw_d = nc.declare_dram_parameter("clf_w", [D], dt.float32, isOutput=False)
    cb_d = nc.declare_dram_parameter("clf_b", [1], dt.float32, isOutput=False)
    out_d = nc.declare_dram_parameter("out", [iters, 1], dt.float32, isOutput=True)

    with tile.TileContext(nc) as tc:
        with tc.tile_pool(name="const", bufs=1) as cpool:
            ident = cpool.tile([P, P], dt.float32, tag="ident", name="ident")
            make_identity(nc, ident[:])
            identb = cpool.tile([P, P], dt.bfloat16, tag="identb", name="identb")
            nc.vector.tensor_copy(identb[:], ident[:])
            for it in range(iters):
                _body(nc, tc, mybir, dt, sdt, (ident, identb),
                      x_d, emb_d, wq_d, wk_d, wv_d, lw_d, lb_d, cw_d, cb_d,
                      out_d.ap()[it:it + 1, :])

    nc.compile()
    return nc


def _body(nc, tc, mybir, dt, sdt, idents,
          x_d, emb_d, wq_d, wk_d, wv_d, lw_d, lb_d, cw_d, cb_d, out_ap):
    import concourse.bass as bass

    AF = mybir.ActivationFunctionType
    AX = mybir.AxisListType
    ALU = mybir.AluOpType
    ident, identb = idents
    bf = dt.bfloat16
    f32 = dt.float32
    f32r = dt.float32r
    use_mix = sdt == "mix"
    use_dr = (not use_mix) and sdt == dt.float8e4
    DR = mybir.MatmulPerfMode.DoubleRow

    # round-robin PSUM->SBUF copies over DVE / ACT
    _cp = [0]

    def copy_ps(out, in_):
        if _cp[0] % 2 == 0:
            nc.vector.tensor_copy(out, in_)
        else:
            nc.scalar.copy(out, in_)
        _cp[0] += 1

    with tc.tile_pool(name="persist", bufs=1) as pp:
        # E^T: [p_f, fc, t] with f = fc*128 + p_f
        E_all = pp.tile([P, DC, T], bf, tag="eall", name="eall")
        # W^T per d-chunk (stationary): [p_f, fc, d]
        WqT = [pp.tile([P, DC, P], bf, tag=f"wqT{d}", name=f"wqT{d}") for d in range(DC)]
        WkT = [pp.tile([P, DC, P], bf, tag=f"wkT{d}", name=f"wkT{d}") for d in range(DC)]
        LwT = [pp.tile([P, DC, P], bf, tag=f"lwT{d}", name=f"lwT{d}") for d in range(DC)]
        # Wv^T as moving: [p_f, fc, d-full]
        WvT = pp.tile([P, DC, D], bf, tag="wvT", name="wvT")
        if use_mix:
            Qb = pp.tile([P, 2, T], bf, tag="qb", name="qb")
            Kb = pp.tile([P, 2, T], bf, tag="kb", name="kb")
            Q8 = pp.tile([P, 2, T], dt.float8e4, tag="q8", name="q8")
            K8 = pp.tile([P, 2, T], dt.float8e4, tag="k8", name="k8")

            mixlo = os.environ.get("KERNEL_MIX_PAIR", "23") == "01"

            def q_dst(qa, dd):
                b, e8 = (Qb, Q8) if qa == 0 else (Kb, K8)
                if mixlo:
                    return e8[:, dd, :] if dd < 2 else b[:, dd - 2, :]
                return b[:, dd, :] if dd < 2 else e8[:, dd - 2, :]
        else:
            Q_all = pp.tile([P, DC, T], sdt, tag="qall", name="qall")
            K_all = pp.tile([P, DC, T], sdt, tag="kall", name="kall")

            def q_dst(qa, dd):
                return (Q_all if qa == 0 else K_all)[:, dd, :]
        V_full = [pp.tile([P, D], bf, tag=f"v{c}", name=f"v{c}") for c in range(TC)]
        linb = pp.tile([P, DC], f32, tag="linb", name="linb")
        clfw = pp.tile([P, DC], f32, tag="clfw", name="clfw")
        clfb = pp.tile([1, 1], f32, tag="clfb", name="clfb")
        zsum = [pp.tile([P, TC], f32, tag=f"zs{d}", name=f"zs{d}") for d in range(DC)]

        # ---------------- setup: gather, transposes, QKV projections ------
        with tc.tile_pool(name="wld", bufs=2) as wp, \
             tc.tile_pool(name="etf_pool", bufs=6) as efp, \
             tc.tile_pool(name="sidx", bufs=1) as sip, \
             tc.tile_pool(name="tp_ps", bufs=4, space="PSUM") as spp, \
             tc.tile_pool(name="qkv_ps", bufs=4, space="PSUM") as qpp:

            idx = sip.tile([P, TC], dt.int32, tag="idx", name="idx")
            nc.sync.dma_start(out=idx[:], in_=x_d.ap().rearrange("(p c) -> p c", c=TC))

            def transpose_w(w_dram, write_out):
                """PE-transpose one DxD weight (bf16), write_out(d2, tp_psum)."""
                wsb = [wp.tile([P, D], f32, tag=f"wL{d2}", name=f"wL{d2}", bufs=2)
                       for d2 in range(DC)]
                for d2 in range(DC):
                    nc.sync.dma_start(out=wsb[d2][:],
                                      in_=w_dram.ap()[d2 * P:(d2 + 1) * P, :])
                for d2 in range(DC):
                    tp = spp.tile([P, DC, P], f32, tag="tp", name="wtp")
                    for fc in range(DC):
                        nc.tensor.transpose(tp[:, fc, :],
                                            wsb[d2][:, fc * P:(fc + 1) * P],
                                            ident[:])
                    write_out(d2, tp)

            transpose_w(wq_d, lambda d2, tp: copy_ps(WqT[d2][:], tp[:]))
            transpose_w(wk_d, lambda d2, tp: copy_ps(WkT[d2][:], tp[:]))
            transpose_w(wv_d, lambda d2, tp: copy_ps(WvT[:, :, d2 * P:(d2 + 1) * P], tp[:]))
            transpose_w(lw_d, lambda d2, tp: copy_ps(LwT[d2][:], tp[:]))

            def gather_group(g):
                etf = [efp.tile([P, D], bf, tag="etf", name="etf") for _ in range(4)]
                for s in range(4):
                    c = g * 4 + s
                    nc.gpsimd.indirect_dma_start(
                        out=etf[s][:],
                        out_offset=None,
                        in_=emb_d.ap(),
                        in_offset=bass.IndirectOffsetOnAxis(ap=idx[:, c:c + 1], axis=0),
                    )
                for s in range(4):
                    c = g * 4 + s
                    tp = spp.tile([P, DC, P], bf, tag="tp", name="etp")
                    for fc in range(DC):
                        nc.tensor.transpose(tp[:, fc, :],
                                            etf[s][:, fc * P:(fc + 1) * P],
                                            identb[:])
                    copy_ps(E_all[:, :, c * P:(c + 1) * P], tp[:])

            for g in range(TC // 4):
                gather_group(g)
                tb = g
                for qa, wT in enumerate((WqT, WkT)):
                    for dd in range(DC):
                        ps = qpp.tile([P, 512], f32, tag="qkv", name="qkv")
                        for fc in range(DC):
                            nc.tensor.matmul(ps[:],
                                             wT[dd][:, fc, :],
                                             E_all[:, fc, tb * 512:(tb + 1) * 512],
                                             start=(fc == 0), stop=(fc == DC - 1))
                        copy_ps(q_dst(qa, dd)[:, tb * 512:(tb + 1) * 512], ps[:])
                for ci in range(4):
                    c = g * 4 + ci
                    ps = qpp.tile([P, 512], f32, tag="qkv", name="qkv")
                    for fc in range(DC):
                        nc.tensor.matmul(ps[:],
                                         E_all[:, fc, c * P:(c + 1) * P],
                                         WvT[:, fc, :],
                                         start=(fc == 0), stop=(fc == DC - 1))
                    copy_ps(V_full[c][:], ps[:])

            nc.sync.dma_start(out=linb[:], in_=lb_d.ap().rearrange("(c p) -> p c", p=P))
            nc.sync.dma_start(out=clfw[:], in_=cw_d.ap().rearrange("(c p) -> p c", p=P))
            nc.sync.dma_start(out=clfb[:], in_=cb_d.ap().unsqueeze(1))

        # ---------------- attention + linear + mean ----------------
        with tc.tile_pool(name="s_ps", bufs=6, space="PSUM") as sps, \
             tc.tile_pool(name="y_ps", bufs=1, space="PSUM") as yps, \
             tc.tile_pool(name="z_ps", bufs=1, space="PSUM") as zps, \
             tc.tile_pool(name="pex", bufs=3) as ppb, \
             tc.tile_pool(name="pt", bufs=3) as ptp, \
             tc.tile_pool(name="ybuf", bufs=2) as ybp, \
             tc.tile_pool(name="ytb", bufs=2) as ytp, \
             tc.tile_pool(name="zr", bufs=1) as zrp, \
             tc.tile_pool(name="scr", bufs=4) as scr:

            zrt = zrp.tile([P, P], bf, tag="zrt", name="zrt")

            st_s = {}    # ic -> [S0..S3] psum tiles
            st_p = {}    # ic -> (PT, linv)
            st_y = {}    # ic -> yT

            def stage_scores(ic):
                Sb = [sps.tile([P, 512], f32, tag="s", name="s") for _ in range(NB)]
                for jb in range(NB):
                    js = slice(jb * 512, (jb + 1) * 512)
                    isl = slice(ic * P, (ic + 1) * P)
                    if use_dr:
                        for h in range(2):
                            nc.tensor.matmul(
                                Sb[jb][:],
                                Q_all[:, 2 * h:2 * h + 2, isl],
                                K_all[:, 2 * h:2 * h + 2, js],
                                start=(h == 0), stop=(h == 1), perf_mode=DR)
                    elif use_mix:
                        for dd in range(2):
                            nc.tensor.matmul(Sb[jb][:], Qb[:, dd, isl],
                                             Kb[:, dd, js],
                                             start=(dd == 0), stop=False)
                        nc.tensor.matmul(Sb[jb][:], Q8[:, :, isl], K8[:, :, js],
                                         start=False, stop=True, perf_mode=DR)
                    else:
                        for dd in range(DC):
                            nc.tensor.matmul(
                                Sb[jb][:],
                                Q_all[:, dd, isl],
                                K_all[:, dd, js],
                                start=(dd == 0), stop=(dd == DC - 1))
                st_s[ic] = Sb

            def stage_softmax(ic):
                Sb = st_s.pop(ic)
                mx4 = scr.tile([P, NB], f32, tag="mx4", name="mx4")
                for jb in range(NB):
                    nc.vector.tensor_reduce(mx4[:, jb:jb + 1], Sb[jb][:],
                                            axis=AX.X, op=ALU.max)
                mx = scr.tile([P, 1], f32, tag="mx", name="mx")
                nc.vector.tensor_reduce(mx[:], mx4[:], axis=AX.X, op=ALU.max)
                negb = scr.tile([P, 1], f32, tag="negb", name="negb")
                nc.vector.tensor_scalar_mul(negb[:], mx[:], -float(SCALE))
                lp = scr.tile([P, NB], f32, tag="lp", name="lp")
                Pex = ppb.tile([P, T], bf, tag="pex", name="pex")
                for jb in range(NB):
                    nc.scalar.activation(Pex[:, jb * 512:(jb + 1) * 512],
                                         Sb[jb][:], AF.Exp,
                                         bias=negb[:], scale=float(SCALE),
                                         accum_out=lp[:, jb:jb + 1])
                lsum = scr.tile([P, 1], f32, tag="lsum", name="lsum")
                nc.vector.tensor_reduce(lsum[:], lp[:], axis=AX.X, op=ALU.add)
                linv = scr.tile([P, 1], f32, tag="linv", name="linv")
                nc.vector.reciprocal(linv[:], lsum[:])
                PT = ptp.tile([P, TC, P], bf, tag="pt", name="pt")
                nc.sync.dma_start_transpose(out=PT[:], in_=Pex[:])
                st_p[ic] = (PT, linv)

            def stage_pv(ic):
                PT, linv = st_p.pop(ic)
                yp = yps.tile([P, D], f32, tag="yp", name="yp")
                for jc in range(TC):
                    nc.tensor.matmul(yp[:], PT[:, jc, :], V_full[jc][:],
                                     start=(jc == 0), stop=(jc == TC - 1))
                y16 = ybp.tile([P, D], bf, tag="y16", name="y16")
                nc.vector.tensor_scalar_mul(y16[:], yp[:], linv[:])
                yT = ytp.tile([P, DC, P], bf, tag="yt", name="yt")
                nc.sync.dma_start_transpose(out=yT[:], in_=y16[:])
                st_y[ic] = yT

            def stage_linear(ic):
                yT = st_y.pop(ic)
                zp = zps.tile([P, DC, P], f32, tag="zp", name="zp")
                for do in range(DC):
                    for dc_ in range(DC):
                        nc.tensor.matmul(zp[:, do, :],
                                         LwT[do][:, dc_, :],
                                         yT[:, dc_, :],
                                         start=(dc_ == 0), stop=(dc_ == DC - 1))
                for do in range(DC):
                    nc.scalar.activation(zrt[:], zp[:, do, :], AF.Relu,
                                         bias=linb[:, do:do + 1], scale=1.0,
                                         accum_out=zsum[do][:, ic:ic + 1])

            for it_ in range(TC + 3):
                if it_ < TC:
                    stage_scores(it_)
                    stage_softmax(it_)
                if 2 <= it_ <= TC + 1:
                    stage_pv(it_ - 2)
                if it_ >= 3:
                    stage_linear(it_ - 3)

            # ---------------- classifier ----------------
            ysumt = scr.tile([P, DC], f32, tag="ysum", name="ysum")
            for do in range(DC):
                nc.vector.tensor_reduce(ysumt[:, do:do + 1], zsum[do][:],
                                        axis=AX.X, op=ALU.add)
            op = zps.tile([P, DC, P], f32, tag="zp", name="op")
            for do in range(DC):
                nc.tensor.matmul(op[:1, 0, :1], clfw[:, do:do + 1],
                                 ysumt[:, do:do + 1],
                                 start=(do == 0), stop=(do == DC - 1))
            osb = scr.tile([1, 1], f32, tag="osb", name="osb")
            nc.scalar.activation(osb[:], op[:1, 0, :1], AF.Sigmoid,
                                 bias=clfb[:], scale=float(1.0 / T))
            nc.sync.dma_start(out=out_ap, in_=osb[:])


def _get_nc(iters=1, mm_dtype=None):
    sdt_name = os.environ.get("KERNEL_SCORES_DTYPE", "f8")
    key = (iters, sdt_name)
    if key not in _COMPILED:
        _COMPILED[key] = _build(iters=iters, sdt_name=sdt_name)
    return _COMPILED[key]


def _in_maps(x, embed, W_q, W_k, W_v, lin_w, lin_b, clf_w, clf_b):
    x = np.ascontiguousarray(np.asarray(x).astype(np.int32))
    common = {
        "embed": np.ascontiguousarray(np.asarray(embed, np.float32)),
        "W_q": np.ascontiguousarray(np.asarray(W_q, np.float32)),
        "W_k": np.ascontiguousarray(np.asarray(W_k, np.float32)),
        "W_v": np.ascontiguousarray(np.asarray(W_v, np.float32)),
        "lin_w": np.ascontiguousarray(np.asarray(lin_w, np.float32)),
        "lin_b": np.ascontiguousarray(np.asarray(lin_b, np.float32).reshape(D)),
        "clf_w": np.ascontiguousarray(np.asarray(clf_w, np.float32).reshape(D)),
        "clf_b": np.ascontiguousarray(np.asarray(clf_b, np.float32).reshape(1)),
    }
    return [dict(common, x=x[c]) for c in range(N_CORES)]


def kernel(x, embed, W_q, W_k, W_v, lin_w, lin_b, clf_w, clf_b):
    from concourse.bass_utils import run_bass_kernel_spmd

    nc = _get_nc()
    in_maps = _in_maps(x, embed, W_q, W_k, W_v, lin_w, lin_b, clf_w, clf_b)
    res = run_bass_kernel_spmd(nc, in_maps, core_ids=list(range(N_CORES)))
    out = np.stack([res.results[c]["out"][0, 0] for c in range(N_CORES)])
    return out.reshape(B, 1).astype(np.float32)


# revision 23
# speedup vs baseline: 1.0925x; 1.0925x over previous
"""BasicTransformer Trainium2 kernel (Bass/Tile), data-parallel over batch on 8 cores.

Per batch b (one NeuronCore each):
    e   = embed[x[b]]                    (T, D)   indirect-DMA gather
    e^T, W^T via PE transposes (f32r) -> bf16 SBUF tiles
    q/k = W^T-stationary matmuls         PE bf16 -> fp8e4 (scores operands)
    v   = E^T-stationary matmuls         PE bf16, [t-part, d] layout
    s   = (q^T k)                        PE fp8 DoubleRow (2 k-subtiles/pass)
    p   = exp(s*SCALE - max*SCALE)       DVE rowmax + ACT exp (accum -> l)
    p^T via DMA-XBAR transpose           -> PT tiles [j-part, jc, i] bf16
    y   = (p @ v) * (1/l)                PE (PT stationary, V moving) +
                                         per-partition tensor_scalar (Pool)
    y^T via DMA-XBAR                     -> linear in [o-part, t] layout
    z   = relu(lw y + b), accum over t   PE + ACT
    out = sigmoid(clf . mean + clf_b)    PE + ACT

The attention inner loop is software-pipelined per 128-query chunk with
pv lagging scores by 2 and the linear by 3 chunks, keeping the PE dense
while the DVE/ACT softmax chain and the XBAR transposes run in the
shadow.  t-order inside the kernel is a fixed permutation of the true
t-order; the computation is permutation-invariant over t, so the final
(1,) output is unaffected.
"""

import math
import os

import numpy as np

B, T, D, VOCAB = 8, 2048, 512, 32000
P = 128
TC = T // P          # 16 t-chunks
DC = D // P          # 4 d-chunks
NB = T // 512        # 4 key blocks
SCALE = 1.0 / math.sqrt(D)
N_CORES = 8

_COMPILED = {}


def _build(iters=1, sdt_name=None):
    import concourse.bacc as bacc
    import concourse.mybir as mybir
    import concourse.tile as tile
    from concourse.masks import make_identity

    dt = mybir.dt
    if sdt_name is None:
        sdt_name = os.environ.get("KERNEL_SCORES_DTYPE", "mix")
    sdt = {"f8": dt.float8e4, "bf16": dt.bfloat16, "mix": "mix"}[sdt_name]

    nc = bacc.Bacc("TRN2", target_bir_lowering=False, debug=False)

    x_d = nc.declare_dram_parameter("x", [T], dt.int32, isOutput=False)
    emb_d = nc.declare_dram_parameter("embed", [VOCAB + 1, D], dt.float32, isOutput=False)
    wq_d = nc.declare_dram_parameter("W_q", [D, D], dt.float32, isOutput=False)
    wk_d = nc.declare_dram_parameter("W_k", [D, D], dt.float32, isOutput=False)
    wv_d = nc.declare_dram_parameter("W_v", [D, D], dt.float32, isOutput=False)
    lw_d = nc.declare_dram_parameter("lin_w", [D, D], dt.float32, isOutput=False)
    lb_d = nc.declare_dram_parameter("lin_b", [D], dt.float32, isOutput=False)
    cw_d = nc.declare_dram_parameter("clf_w", [D], dt.float32, isOutput=False)
    cb_d = nc.declare_dram_parameter("clf_b", [1], dt.float32, isOutput=False)
    out_d = nc.declare_dram_parameter("out", [iters, 1], dt.float32, isOutput=True)

    with tile.TileContext(nc) as tc:
        with tc.tile_pool(name="const", bufs=1) as cpool:
            ident = cpool.tile([P, P], dt.float32, tag="ident", name="ident")
            make_identity(nc, ident[:])
            identb = cpool.tile([P, P], dt.bfloat16, tag="identb", name="identb")
            nc.vector.tensor_copy(identb[:], ident[:])
            for it in range(iters):
                _body(nc, tc, mybir, dt, sdt, (ident, identb),
                      x_d, emb_d, wq_d, wk_d, wv_d, lw_d, lb_d, cw_d, cb_d,
                      out_d.ap()[it:it + 1, :])

    nc.compile()
    return nc


def _body(nc, tc, mybir, dt, sdt, idents,
          x_d, emb_d, wq_d, wk_d, wv_d, lw_d, lb_d, cw_d, cb_d, out_ap):
    import concourse.bass as bass

    AF = mybir.ActivationFunctionType
    AX = mybir.AxisListType
    ALU = mybir.AluOpType
    ident, identb = idents
    bf = dt.bfloat16
    f32 = dt.float32
    f32r = dt.float32r
    use_mix = sdt == "mix"
    use_dr = (not use_mix) and sdt == dt.float8e4
    DR = mybir.MatmulPerfMode.DoubleRow

    # round-robin PSUM->SBUF copies over DVE / ACT
    _cp = [0]

    def copy_ps(out, in_):
        if _cp[0] % 2 == 0:
            nc.vector.tensor_copy(out, in_)
        else:
            nc.scalar.copy(out, in_)
        _cp[0] += 1

    with tc.tile_pool(name="persist", bufs=1) as pp:
        # E^T: [p_f, fc, t] with f = fc*128 + p_f
        E_all = pp.tile([P, DC, T], bf, tag="eall", name="eall")
        # W^T per d-chunk (stationary): [p_f, fc, d]
        WqT = [pp.tile([P, DC, P], bf, tag=f"wqT{d}", name=f"wqT{d}") for d in range(DC)]
        WkT = [pp.tile([P, DC, P], bf, tag=f"wkT{d}", name=f"wkT{d}") for d in range(DC)]
        LwT = [pp.tile([P, DC, P], bf, tag=f"lwT{d}", name=f"lwT{d}") for d in range(DC)]
        # Wv^T as moving: [p_f, fc, d-full]
        WvT = pp.tile([P, DC, D], bf, tag="wvT", name="wvT")
        if use_mix:
            Qb = pp.tile([P, 2, T], bf, tag="qb", name="qb")
            Kb = pp.tile([P, 2, T], bf, tag="kb", name="kb")
            Q8 = pp.tile([P, 2, T], dt.float8e4, tag="q8", name="q8")
            K8 = pp.tile([P, 2, T], dt.float8e4, tag="k8", name="k8")

            mixlo = os.environ.get("KERNEL_MIX_PAIR", "01") == "01"

            def q_dst(qa, dd):
                b, e8 = (Qb, Q8) if qa == 0 else (Kb, K8)
                if mixlo:
                    return e8[:, dd, :] if dd < 2 else b[:, dd - 2, :]
                return b[:, dd, :] if dd < 2 else e8[:, dd - 2, :]
        else:
            Q_all = pp.tile([P, DC, T], sdt, tag="qall", name="qall")
            K_all = pp.tile([P, DC, T], sdt, tag="kall", name="kall")

            def q_dst(qa, dd):
                return (Q_all if qa == 0 else K_all)[:, dd, :]
        V_full = [pp.tile([P, D], bf, tag=f"v{c}", name=f"v{c}") for c in range(TC)]
        linb = pp.tile([P, DC], f32, tag="linb", name="linb")
        clfw = pp.tile([P, DC], f32, tag="clfw", name="clfw")
        clfb = pp.tile([1, 1], f32, tag="clfb", name="clfb")
        zsum = [pp.tile([P, TC], f32, tag=f"zs{d}", name=f"zs{d}") for d in range(DC)]

        # ---------------- setup: gather, transposes, QKV projections ------
        with tc.tile_pool(name="wld", bufs=2) as wp, \
             tc.tile_pool(name="etf_pool", bufs=6) as efp, \
             tc.tile_pool(name="sidx", bufs=1) as sip, \
             tc.tile_pool(name="tp_ps", bufs=4, space="PSUM") as spp, \
             tc.tile_pool(name="qkv_ps", bufs=4, space="PSUM") as qpp:

            idx = sip.tile([P, TC], dt.int32, tag="idx", name="idx")
            nc.sync.dma_start(out=idx[:], in_=x_d.ap().rearrange("(p c) -> p c", c=TC))

            def transpose_w(w_dram, write_out):
                """PE-transpose one DxD weight, write_out(d2, tp_psum)."""
                wsb = [wp.tile([P, D], f32, tag=f"wL{d2}", name=f"wL{d2}", bufs=2)
                       for d2 in range(DC)]
                for d2 in range(DC):
                    nc.sync.dma_start(out=wsb[d2][:],
                                      in_=w_dram.ap()[d2 * P:(d2 + 1) * P, :])
                for d2 in range(DC):
                    tp = spp.tile([P, DC, P], f32, tag="tp", name="wtp")
                    for fc in range(DC):
                        nc.tensor.transpose(tp[:, fc, :],
                                            wsb[d2][:, fc * P:(fc + 1) * P],
                                            ident[:])
                    write_out(d2, tp)

            transpose_w(wq_d, lambda d2, tp: copy_ps(WqT[d2][:], tp[:]))
            transpose_w(wk_d, lambda d2, tp: copy_ps(WkT[d2][:], tp[:]))
            transpose_w(wv_d, lambda d2, tp: copy_ps(WvT[:, :, d2 * P:(d2 + 1) * P], tp[:]))
            transpose_w(lw_d, lambda d2, tp: copy_ps(LwT[d2][:], tp[:]))

            def gather_group(g):
                etf = [efp.tile([P, D], bf, tag="etf", name="etf") for _ in range(4)]
                for s in range(4):
                    c = g * 4 + s
                    nc.gpsimd.indirect_dma_start(
                        out=etf[s][:],
                        out_offset=None,
                        in_=emb_d.ap(),
                        in_offset=bass.IndirectOffsetOnAxis(ap=idx[:, c:c + 1], axis=0),
                    )
                for s in range(4):
                    c = g * 4 + s
                    tp = spp.tile([P, DC, P], bf, tag="tp", name="etp")
                    for fc in range(DC):
                        nc.tensor.transpose(tp[:, fc, :],
                                            etf[s][:, fc * P:(fc + 1) * P],
                                            identb[:])
                    copy_ps(E_all[:, :, c * P:(c + 1) * P], tp[:])

            for g in range(TC // 4):
                gather_group(g)
                tb = g
                worder = ((0, WqT), (1, WkT)) if g < 3 else ((1, WkT), (0, WqT))
                for qa, wT in worder:
                    for dd in range(DC):
                        ps = qpp.tile([P, 512], f32, tag="qkv", name="qkv")
                        for fc in range(DC):
                            nc.tensor.matmul(ps[:],
                                             wT[dd][:, fc, :],
                                             E_all[:, fc, tb * 512:(tb + 1) * 512],
                                             start=(fc == 0), stop=(fc == DC - 1))
                        copy_ps(q_dst(qa, dd)[:, tb * 512:(tb + 1) * 512], ps[:])
                for ci in range(4):
                    c = g * 4 + ci
                    ps = qpp.tile([P, 512], f32, tag="qkv", name="qkv")
                    for fc in range(DC):
                        nc.tensor.matmul(ps[:],
                                         E_all[:, fc, c * P:(c + 1) * P],
                                         WvT[:, fc, :],
                                         start=(fc == 0), stop=(fc == DC - 1))
                    copy_ps(V_full[c][:], ps[:])

            nc.sync.dma_start(out=linb[:], in_=lb_d.ap().rearrange("(c p) -> p c", p=P))
            nc.sync.dma_start(out=clfw[:], in_=cw_d.ap().rearrange("(c p) -> p c", p=P))
            nc.sync.dma_start(out=clfb[:], in_=cb_d.ap().unsqueeze(1))

        # ---------------- attention + linear + mean ----------------
        with tc.tile_pool(name="s_ps", bufs=6, space="PSUM") as sps, \
             tc.tile_pool(name="y_ps", bufs=1, space="PSUM") as yps, \
             tc.tile_pool(name="z_ps", bufs=1, space="PSUM") as zps, \
             tc.tile_pool(name="pex", bufs=3) as ppb, \
             tc.tile_pool(name="pt", bufs=4) as ptp, \
             tc.tile_pool(name="ybuf", bufs=3) as ybp, \
             tc.tile_pool(name="ytb", bufs=3) as ytp, \
             tc.tile_pool(name="zr", bufs=1) as zrp, \
             tc.tile_pool(name="scr", bufs=5) as scr:

            zrt = zrp.tile([P, P], bf, tag="zrt", name="zrt")

            st_s = {}    # ic -> [S0..S3] psum tiles
            st_p = {}    # ic -> (PT, linv)
            st_y = {}    # ic -> yT

            def stage_scores(ic):
                Sb = [sps.tile([P, 512], f32, tag="s", name="s") for _ in range(NB)]
                for jb in range(NB):
                    js = slice(jb * 512, (jb + 1) * 512)
                    isl = slice(ic * P, (ic + 1) * P)
                    if use_dr:
                        for h in range(2):
                            nc.tensor.matmul(
                                Sb[jb][:],
                                Q_all[:, 2 * h:2 * h + 2, isl],
                                K_all[:, 2 * h:2 * h + 2, js],
                                start=(h == 0), stop=(h == 1), perf_mode=DR)
                    elif use_mix:
                        for dd in range(2):
                            nc.tensor.matmul(Sb[jb][:], Qb[:, dd, isl],
                                             Kb[:, dd, js],
                                             start=(dd == 0), stop=False)
                        nc.tensor.matmul(Sb[jb][:], Q8[:, :, isl], K8[:, :, js],
                                         start=False, stop=True, perf_mode=DR)
                    else:
                        for dd in range(DC):
                            nc.tensor.matmul(
                                Sb[jb][:],
                                Q_all[:, dd, isl],
                                K_all[:, dd, js],
                                start=(dd == 0), stop=(dd == DC - 1))
                st_s[ic] = Sb

            def stage_softmax(ic):
                Sb = st_s.pop(ic)
                mx4 = scr.tile([P, NB], f32, tag="mx4", name="mx4")
                for jb in range(NB):
                    nc.vector.tensor_reduce(mx4[:, jb:jb + 1], Sb[jb][:],
                                            axis=AX.X, op=ALU.max)
                mx = scr.tile([P, 1], f32, tag="mx", name="mx")
                nc.vector.tensor_reduce(mx[:], mx4[:], axis=AX.X, op=ALU.max)
                negb = scr.tile([P, 1], f32, tag="negb", name="negb")
                nc.vector.tensor_scalar_mul(negb[:], mx[:], -float(SCALE))
                lp = scr.tile([P, NB], f32, tag="lp", name="lp")
                Pex = ppb.tile([P, T], bf, tag="pex", name="pex")
                for jb in range(NB):
                    nc.scalar.activation(Pex[:, jb * 512:(jb + 1) * 512],
                                         Sb[jb][:], AF.Exp,
                                         bias=negb[:], scale=float(SCALE),
                                         accum_out=lp[:, jb:jb + 1])
                lsum = scr.tile([P, 1], f32, tag="lsum", name="lsum")
                nc.vector.tensor_reduce(lsum[:], lp[:], axis=AX.X, op=ALU.add)
                linv = scr.tile([P, 1], f32, tag="linv", name="linv")
                nc.vector.reciprocal(linv[:], lsum[:])
                PT = ptp.tile([P, TC, P], bf, tag="pt", name="pt")
                nc.sync.dma_start_transpose(out=PT[:], in_=Pex[:])
                st_p[ic] = (PT, linv)

            def stage_pv(ic):
                PT, linv = st_p.pop(ic)
                yp = yps.tile([P, D], f32, tag="yp", name="yp")
                for jc in range(TC):
                    nc.tensor.matmul(yp[:], PT[:, jc, :], V_full[jc][:],
                                     start=(jc == 0), stop=(jc == TC - 1))
                y16 = ybp.tile([P, D], bf, tag="y16", name="y16")
                nc.vector.tensor_scalar_mul(y16[:], yp[:], linv[:])
                yT = ytp.tile([P, DC, P], bf, tag="yt", name="yt")
                nc.sync.dma_start_transpose(out=yT[:], in_=y16[:])
                st_y[ic] = yT

            def stage_linear(ic):
                yT = st_y.pop(ic)
                zp = zps.tile([P, DC, P], f32, tag="zp", name="zp")
                for do in range(DC):
                    for dc_ in range(DC):
                        nc.tensor.matmul(zp[:, do, :],
                                         LwT[do][:, dc_, :],
                                         yT[:, dc_, :],
                                         start=(dc_ == 0), stop=(dc_ == DC - 1))
                for do in range(DC):
                    nc.scalar.activation(zrt[:], zp[:, do, :], AF.Relu,
                                         bias=linb[:, do:do + 1], scale=1.0,
                                         accum_out=zsum[do][:, ic:ic + 1])

            for it_ in range(TC + 4):
                if it_ < TC:
                    stage_scores(it_)
                    stage_softmax(it_)
                if 3 <= it_ <= TC + 2:
                    stage_pv(it_ - 3)
                if it_ >= 4:
                    stage_linear(it_ - 4)

            # ---------------- classifier ----------------
            ysumt = scr.tile([P, DC], f32, tag="ysum", name="ysum")
            for do in range(DC):
                nc.vector.tensor_reduce(ysumt[:, do:do + 1], zsum[do][:],
                                        axis=AX.X, op=ALU.add)
            op = zps.tile([P, DC, P], f32, tag="zp", name="op")
            for do in range(DC):
                nc.tensor.matmul(op[:1, 0, :1], clfw[:, do:do + 1],
                                 ysumt[:, do:do + 1],
                                 start=(do == 0), stop=(do == DC - 1))
            osb = scr.tile([1, 1], f32, tag="osb", name="osb")
            nc.scalar.activation(osb[:], op[:1, 0, :1], AF.Sigmoid,
                                 bias=clfb[:], scale=float(1.0 / T))
            nc.sync.dma_start(out=out_ap, in_=osb[:])


def _get_nc(iters=1, mm_dtype=None):
    sdt_name = os.environ.get("KERNEL_SCORES_DTYPE", "f8")
    key = (iters, sdt_name)
    if key not in _COMPILED:
        _COMPILED[key] = _build(iters=iters, sdt_name=sdt_name)
    return _COMPILED[key]


def _in_maps(x, embed, W_q, W_k, W_v, lin_w, lin_b, clf_w, clf_b):
    x = np.ascontiguousarray(np.asarray(x).astype(np.int32))
    common = {
        "embed": np.ascontiguousarray(np.asarray(embed, np.float32)),
        "W_q": np.ascontiguousarray(np.asarray(W_q, np.float32)),
        "W_k": np.ascontiguousarray(np.asarray(W_k, np.float32)),
        "W_v": np.ascontiguousarray(np.asarray(W_v, np.float32)),
        "lin_w": np.ascontiguousarray(np.asarray(lin_w, np.float32)),
        "lin_b": np.ascontiguousarray(np.asarray(lin_b, np.float32).reshape(D)),
        "clf_w": np.ascontiguousarray(np.asarray(clf_w, np.float32).reshape(D)),
        "clf_b": np.ascontiguousarray(np.asarray(clf_b, np.float32).reshape(1)),
    }
    return [dict(common, x=x[c]) for c in range(N_CORES)]


def kernel(x, embed, W_q, W_k, W_v, lin_w, lin_b, clf_w, clf_b):
    from concourse.bass_utils import run_bass_kernel_spmd

    nc = _get_nc()
    in_maps = _in_maps(x, embed, W_q, W_k, W_v, lin_w, lin_b, clf_w, clf_b)
    res = run_bass_kernel_spmd(nc, in_maps, core_ids=list(range(N_CORES)))
    out = np.stack([res.results[c]["out"][0, 0] for c in range(N_CORES)])
    return out.reshape(B, 1).astype(np.float32)
